# revision 2
# baseline (speedup 1.0000x reference)
"""Trainium2 Bass kernel for nn_MixtureOfExpertsHead — optimized dense version.

Baseline structure (data-parallel, 4 passes of 512 tokens, dense all-expert
compute) with two changes:
- Gate runs as a 3-term bf16 split (x=hi+lo, W=hi+lo; xh@Wh + xh@Wl + xl@Wh,
  fp32 PSUM): 3 cycles/row instead of fp32's 4, zero top-2 selection flips
  measured vs fp32 on this dataset. x streams as two bf16 tensors (hi/lo);
  the hi part doubles as the expert-matmul input.
- All weight DMAs are single fully-contiguous 1MB transfers per (expert,
  m-chunk) instead of 4x strided 256KB slices (HBM-unfriendly stride
  pattern was the main baseline bottleneck suspect).
"""

import sys

sys.path.insert(0, "/opt/trn_rl_repo")

import ml_dtypes
import numpy as np

import concourse.bacc as bacc
import concourse.mybir as mybir
import concourse.tile as tile
from concourse.bass_utils import run_bass_kernel_spmd
from concourse.masks import make_identity

B, H, E, KTOP, OD = 16384, 4096, 8, 2, 1
H2 = H // 2
NCORES = 8
TOK = B // NCORES      # 2048
TT = 512               # tokens per pass
NP = TOK // TT         # 4 passes
KC = H // 128          # 32
MC = H2 // 128         # 16
NSUB = TT // 128       # 4

f32 = mybir.dt.float32
bf16 = mybir.dt.bfloat16
AF = mybir.ActivationFunctionType
AX = mybir.AxisListType
ALU = mybir.AluOpType


def _build(rep: int = 1):
    nc = bacc.Bacc()
    dp = nc.declare_dram_parameter
    xThif = dp("xThif", [NP, 128, KC, TT], bf16, isOutput=False)
    xTlof = dp("xTlof", [NP, 128, KC, TT], bf16, isOutput=False)
    Wg1hi = dp("Wg1hi", [MC, 128, KC, 128], bf16, isOutput=False)
    Wg1lo = dp("Wg1lo", [MC, 128, KC, 128], bf16, isOutput=False)
    We1c = dp("We1c", [E, MC, 128, KC, 128], bf16, isOutput=False)
    Wg2r = dp("Wg2r", [128, MC, E], f32, isOutput=False)
    We2p = dp("We2p", [128, MC, E, E], bf16, isOutput=False)
    be1r = dp("be1r", [128, E, MC], f32, isOutput=False)
    bg1r = dp("bg1r", [128, MC], f32, isOutput=False)
    bg2b = dp("bg2b", [128, E], f32, isOutput=False)
    be2r = dp("be2r", [E, 1], f32, isOutput=False)
    out = dp("out", [TOK, 1], f32, isOutput=True)

    outr = out.rearrange("(c p) o -> p (c o)", p=128)

    with tile.TileContext(nc) as tc:
        with (
            tc.tile_pool(name="consts", bufs=1) as consts,
            tc.tile_pool(name="xp", bufs=1) as xp,
            tc.tile_pool(name="wgp", bufs=3) as wgp,
            tc.tile_pool(name="wep", bufs=4) as wep,
            tc.tile_pool(name="hp", bufs=3) as hp,
            tc.tile_pool(name="laccp", bufs=2) as laccp,
            tc.tile_pool(name="wtp", bufs=2) as wtp,
            tc.tile_pool(name="eosbp", bufs=2) as eosbp,
            tc.tile_pool(name="tops", bufs=2) as tops,
            tc.tile_pool(name="outp", bufs=1) as outp,
            tc.tile_pool(name="bigp", bufs=4, space="PSUM") as bigp,
            tc.tile_pool(name="smallp", bufs=2, space="PSUM") as smallp,
            tc.tile_pool(name="eop", bufs=2, space="PSUM") as eop,
        ):
            wg2_sb = consts.tile([128, MC, E], f32)
            nc.sync.dma_start(wg2_sb[:], Wg2r[:])
            we2_sb = consts.tile([128, MC, E, E], bf16)
            nc.sync.dma_start(we2_sb[:], We2p[:])
            be1_sb = consts.tile([128, E, MC], f32)
            nc.sync.dma_start(be1_sb[:], be1r[:])
            bg1_sb = consts.tile([128, MC], f32)
            nc.sync.dma_start(bg1_sb[:], bg1r[:])
            bg2_sb = consts.tile([128, E], f32)
            nc.sync.dma_start(bg2_sb[:], bg2b[:])
            be2_sb = consts.tile([E, 1], f32)
            nc.sync.dma_start(be2_sb[:], be2r[:])
            ident = consts.tile([128, 128], f32)
            make_identity(nc, ident[:])

            out_sb = outp.tile([128, TOK // 128], f32)

            for _r in range(rep):
                for p in range(NP):
                    xh_sb = xp.tile([128, KC, TT], bf16, tag="xh")
                    nc.sync.dma_start(xh_sb[:], xThif[p])
                    xl_sb = xp.tile([128, KC, TT], bf16, tag="xl")
                    nc.sync.dma_start(xl_sb[:], xTlof[p])

                    # ======== GATE (3-term bf16 split) ========
                    lacc = laccp.tile([128, NSUB, E], f32, tag="lacc")
                    prev = None
                    for m in range(MC):
                        wh = wgp.tile([128, KC, 128], bf16, tag="wh")
                        nc.sync.dma_start(wh[:], Wg1hi[m])
                        wl = wgp.tile([128, KC, 128], bf16, tag="wl")
                        nc.sync.dma_start(wl[:], Wg1lo[m])
                        ps = bigp.tile([128, TT], f32, tag="big")
                        first = True
                        for wmat, xmat in ((wh, xh_sb), (wl, xh_sb),
                                           (wh, xl_sb)):
                            for c in range(KC):
                                nc.tensor.matmul(
                                    ps, lhsT=wmat[:, c], rhs=xmat[:, c],
                                    start=first,
                                    stop=(wmat is wh and xmat is xl_sb
                                          and c == KC - 1),
                                )
                                first = False
                        gh = hp.tile([128, TT], f32, tag="gh")
                        nc.scalar.activation(
                            gh[:], ps, AF.Relu, bias=bg1_sb[:, m:m + 1]
                        )
                        if prev is not None:
                            _logits_mms(nc, smallp, prev[0], prev[1],
                                        wg2_sb, lacc, bg2_sb)
                        prev = (gh, m)
                    _logits_mms(nc, smallp, prev[0], prev[1],
                                wg2_sb, lacc, bg2_sb)

                    # ---- top-2 softmax weights ----
                    shp = [128, NSUB, E]
                    m1 = tops.tile([128, NSUB, 1], f32, tag="m1")
                    nc.vector.reduce_max(m1[:], lacc[:], axis=AX.X)
                    d = tops.tile(shp, f32, tag="d")
                    nc.vector.tensor_tensor(
                        d[:], lacc[:], m1[:].to_broadcast(shp), ALU.subtract
                    )
                    eq = tops.tile(shp, f32, tag="eq")
                    nc.vector.tensor_scalar(eq[:], d[:], 0.0, None, ALU.is_ge)
                    masked = tops.tile(shp, f32, tag="masked")
                    nc.vector.scalar_tensor_tensor(
                        masked[:], eq[:], -1e30, d[:], ALU.mult, ALU.add
                    )
                    dm2 = tops.tile([128, NSUB, 1], f32, tag="dm2")
                    nc.vector.reduce_max(dm2[:], masked[:], axis=AX.X)
                    ex = tops.tile(shp, f32, tag="ex")
                    nc.scalar.activation(ex[:], d[:], AF.Exp)
                    mask2 = tops.tile(shp, f32, tag="mask2")
                    nc.vector.tensor_tensor(
                        mask2[:], d[:], dm2[:].to_broadcast(shp), ALU.is_ge
                    )
                    u = tops.tile(shp, f32, tag="u")
                    nc.vector.tensor_tensor(u[:], ex[:], mask2[:], ALU.mult)
                    s = tops.tile([128, NSUB, 1], f32, tag="s")
                    nc.vector.reduce_sum(s[:], u[:], axis=AX.X)
                    rinv = tops.tile([128, NSUB, 1], f32, tag="rinv")
                    nc.vector.reciprocal(rinv[:], s[:])
                    wt = wtp.tile(shp, f32, tag="wt")
                    nc.vector.tensor_tensor(
                        wt[:], u[:], rinv[:].to_broadcast(shp), ALU.mult
                    )

                    # ======== EXPERTS (dense, bf16) ========
                    eo_sb = eosbp.tile([E, TT], f32, tag="eosb")
                    eo_ps = eop.tile([E, TT], f32, tag="eo")
                    for e in range(E):
                        prev = None
                        for m in range(MC):
                            we = wep.tile([128, KC, 128], bf16, tag="we")
                            nc.sync.dma_start(we[:], We1c[e, m])
                            ps = bigp.tile([128, TT], f32, tag="big")
                            for c in range(KC):
                                nc.tensor.matmul(
                                    ps, lhsT=we[:, c], rhs=xh_sb[:, c],
                                    start=(c == 0), stop=(c == KC - 1),
                                )
                            ht = hp.tile([128, TT], bf16, tag="hs")
                            nc.scalar.activation(
                                ht[:], ps, AF.Relu, bias=be1_sb[:, e, m:m + 1]
                            )
                            if prev is not None:
                                _eo_mm(nc, eo_ps, we2_sb, e, prev[1], prev[0])
                            prev = (ht, m)
                        _eo_mm(nc, eo_ps, we2_sb, e, prev[1], prev[0])
                    nc.scalar.activation(
                        eo_sb[:], eo_ps, AF.Identity, bias=be2_sb[0:E, 0:1]
                    )

                    # ======== COMBINE ========
                    for sub in range(NSUB):
                        tp = smallp.tile([128, E], f32, tag="small")
                        nc.tensor.transpose(
                            tp,
                            eo_sb[:, sub * 128:(sub + 1) * 128],
                            ident[0:E, 0:E],
                        )
                        prod = tops.tile([128, E], f32, tag="prod")
                        nc.vector.tensor_tensor(
                            prod[:], tp, wt[:, sub, :], ALU.mult
                        )
                        gcol = p * NSUB + sub
                        nc.vector.reduce_sum(
                            out_sb[:, gcol:gcol + 1], prod[:], axis=AX.X
                        )

                nc.sync.dma_start(outr[:], out_sb[:])

    nc.compile()
    return nc


def _logits_mms(nc, smallp, gh, m, wg2_sb, lacc, bg2_sb):
    for sub in range(NSUB):
        lp = smallp.tile([128, E], f32, tag="small")
        nc.tensor.matmul(
            lp,
            lhsT=gh[:, sub * 128:(sub + 1) * 128],
            rhs=wg2_sb[:, m, :],
            start=True,
            stop=True,
        )
        if m == 0:
            nc.vector.tensor_tensor(lacc[:, sub, :], lp, bg2_sb[:], ALU.add)
        else:
            nc.vector.tensor_tensor(
                lacc[:, sub, :], lacc[:, sub, :], lp, ALU.add
            )


def _eo_mm(nc, eo_ps, we2_sb, e, m, ht):
    nc.tensor.matmul(
        eo_ps,
        lhsT=we2_sb[:, m, e, :],
        rhs=ht[:],
        start=(e == 0 and m == 0),
        stop=(e == E - 1 and m == MC - 1),
    )


_NC_CACHE = {}


def _get_nc(rep: int = 1):
    if rep not in _NC_CACHE:
        _NC_CACHE[rep] = _build(rep)
    return _NC_CACHE[rep]


def _prep_in_maps(inputs):
    bf = ml_dtypes.bfloat16
    x = np.asarray(inputs["x"], dtype=np.float32)
    We1 = np.asarray(inputs["We1"], dtype=np.float32)
    be1 = np.asarray(inputs["be1"], dtype=np.float32)
    We2 = np.asarray(inputs["We2"], dtype=np.float32)
    be2 = np.ascontiguousarray(np.asarray(inputs["be2"], dtype=np.float32))
    Wg1 = np.asarray(inputs["Wg1"], dtype=np.float32)
    bg1 = np.asarray(inputs["bg1"], dtype=np.float32)
    Wg2 = np.asarray(inputs["Wg2"], dtype=np.float32)
    bg2 = np.asarray(inputs["bg2"], dtype=np.float32)

    Wg1h = Wg1.astype(bf)
    Wg1l = (Wg1 - Wg1h.astype(np.float32)).astype(bf)

    def wchunk(w):
        return np.ascontiguousarray(
            w.reshape(KC, 128, MC, 128).transpose(2, 1, 0, 3))

    We1c = np.ascontiguousarray(
        We1.astype(bf).reshape(E, KC, 128, MC, 128).transpose(0, 3, 2, 1, 4))
    Wg2r = np.ascontiguousarray(Wg2.reshape(MC, 128, E).transpose(1, 0, 2))
    We2p = np.zeros((128, MC, E, E), dtype=np.float32)
    for e in range(E):
        We2p[:, :, e, e] = We2[e, :, 0].reshape(MC, 128).T
    We2p = We2p.astype(bf)
    be1r = np.ascontiguousarray(be1.reshape(E, MC, 128).transpose(2, 0, 1))
    bg1r = np.ascontiguousarray(bg1.reshape(MC, 128).T)
    bg2b = np.ascontiguousarray(np.tile(bg2[None, :], (128, 1)))

    shared = {
        "Wg1hi": wchunk(Wg1h), "Wg1lo": wchunk(Wg1l), "We1c": We1c,
        "Wg2r": Wg2r, "We2p": We2p, "be1r": be1r, "bg1r": bg1r,
        "bg2b": bg2b, "be2r": be2,
    }
    in_maps = []
    for cidx in range(NCORES):
        xs = x[cidx * TOK:(cidx + 1) * TOK]
        xh = xs.astype(bf)
        xl = (xs - xh.astype(np.float32)).astype(bf)

        def xchunk(a):
            return np.ascontiguousarray(
                a.reshape(NP, TT, KC, 128).transpose(0, 3, 2, 1))

        m = dict(shared)
        m["xThif"] = xchunk(xh)
        m["xTlof"] = xchunk(xl)
        in_maps.append(m)
    return in_maps


def kernel(**inputs) -> np.ndarray:
    in_maps = _prep_in_maps(inputs)
    nc = _get_nc(rep=1)
    res = run_bass_kernel_spmd(nc, in_maps, list(range(NCORES)))
    return np.concatenate(
        [res.results[c]["out"] for c in range(NCORES)], axis=0
    ).astype(np.float32)
